# revision 1
# baseline (speedup 1.0000x reference)
"""CoaT serial block (ConvPosEnc + FactorAtt-ConvRelPosEnc + MLP) on 8 trn2
NeuronCores, data-parallel over batch (16 -> 2 per core).

Per-core layout strategy:
  - Master activations FEATURE-major: x^T [C=256 (2 part-tiles), N=3137] fp32;
    matmul operands bf16.
  - LayerNorm per-token stats via N=1 matmuls (ones moving operand) giving
    compact [tok%128, tok//128] columns; rstd and -mu*rstd are PE-transposed,
    DMA-gathered to [1, 3200] rows, then gpsimd partition_broadcast.
    rstd = exp(-0.5 * ln(var + eps)) keeps ACT on one table set.
  - Depthwise convs (cpe 3x3 on x; crpe 3/5/7 on v) on the tensor engine as
    32x32 diagonal-block matmuls at all 16 tile positions: 3 partition-rotated
    image copies (DMA), 4 PSUM banks (one per rotation class -- each PSUM
    element region must have a single writing tile position), bank sum on
    gpsimd after DVE evac.
  - softmax(k) over tokens skips the max (|k| small): exp on ACT, sums via
    N=1 matmuls; kv/factor are small per-head matmuls (factor via diagonal
    tile positions).
  - scale=Ch^-0.5 folded into Wq; crpe weights divided by scale; k-bias
    dropped (softmax-invariant); v-bias folded into kv rows + conv bias.
"""

import numpy as np

B, N, C, HEADS = 16, 3137, 256, 8
CH = C // HEADS
HW = 56
NPIX = HW * HW            # 3136
PW = 62                   # padded image width/height (pad=3)
GUARD = 4                 # extra zero cols so tap reads never leave the tile
PADIMG = PW * PW          # 3844
PADBUF = PADIMG + 2 * GUARD
P = 128
NTOK_T = 25
LAST_M = N - 24 * P       # 65
SCALE = CH ** -0.5
RPC = 8                   # image rows per conv chunk
NCHUNK = HW // RPC        # 7
CONV_N = RPC * PW         # 496
CPX = RPC * HW            # 448 valid pixels per conv chunk
SEQ_CHUNKS = [(i * 512, min(512, N - i * 512)) for i in range((N + 511) // 512)]

_COMPILED = {}


# ---------------------------------------------------------------- host prep --

def _grid(K):
    p = K // 2
    return [((dy - p) * PW + (dx - p)) for dy in range(K) for dx in range(K)]


def _pack_diag(wtile, T):
    """wtile [128, T] per-channel per-tap weights -> 4-bank rotated diag pack
    [128, T*32]: for tap t (bank b=t%4) partition block i holds diag weights
    of channel block j=(i+b)%4."""
    out = np.zeros((P, T * 32), np.float32)
    for t in range(T):
        b = t % 4
        for i in range(4):
            j = (i + b) % 4
            out[32 * i:32 * i + 32, 32 * t:32 * t + 32] = np.diag(
                wtile[32 * j:32 * j + 32, t])
    return out


def _prep_consts(w):
    c = {}
    qkv_w, qkv_b = w["qkv_w"], w["qkv_b"]
    c["wqT"] = np.ascontiguousarray((qkv_w[0:256] * SCALE).T).reshape(2, P, 256)
    c["wkvT"] = np.ascontiguousarray(qkv_w[256:768].T).reshape(2, P, 512)
    c["qb"] = (qkv_b[0:256] * SCALE).reshape(2, P, 1)
    vb = qkv_b[512:768]
    c["vb_row"] = np.stack([np.broadcast_to(vb[128 * g:128 * g + 128], (P, P))
                            for g in range(2)]).copy()
    c["projT"] = np.ascontiguousarray(w["proj_w"].T).reshape(2, P, 256)
    c["pb"] = w["proj_b"].reshape(2, P, 1)
    c["fc1T"] = np.ascontiguousarray(w["fc1_w"].T).reshape(2, P, 1024)
    c["f1b"] = w["fc1_b"].reshape(8, P, 1)
    c["fc2T"] = np.ascontiguousarray(w["fc2_w"].T).reshape(8, P, 256)
    c["f2b"] = w["fc2_b"].reshape(2, P, 1)
    for nm in ("ln1_g", "ln1_b", "ln2_g", "ln2_b"):
        c[nm] = w[nm].reshape(2, P, 1)
    c["cpe_b"] = w["cpe_b"].reshape(2, P, 1)
    crpe_b_cat = np.concatenate([w["crpe_b3"], w["crpe_b5"], w["crpe_b7"]])
    wsum = np.concatenate([
        w["crpe_w3"].reshape(64, -1).sum(1),
        w["crpe_w5"].reshape(96, -1).sum(1),
        w["crpe_w7"].reshape(96, -1).sum(1)])
    c["crpe_be"] = ((crpe_b_cat + vb * wsum) / SCALE).reshape(2, P, 1)

    # conv diag packs
    cw = w["cpe_w"][:, 0]                       # [256, 3, 3]
    for ct in range(2):
        wt = cw[ct * P:(ct + 1) * P].reshape(P, 9)
        c[f"cpe_pack{ct}"] = _pack_diag(wt, 9)
    w3 = w["crpe_w3"][:, 0] / SCALE             # [64, 3, 3]
    w5 = w["crpe_w5"][:, 0] / SCALE             # [96, 5, 5]
    w7 = w["crpe_w7"][:, 0] / SCALE             # [96, 7, 7]
    wA = np.zeros((P, 25), np.float32)
    wA[0:64] = np.pad(w3, ((0, 0), (1, 1), (1, 1))).reshape(64, 25)
    wA[64:128] = w5[0:64].reshape(64, 25)
    c["crpeA_pack"] = _pack_diag(wA, 25)
    wB = np.zeros((P, 49), np.float32)
    wB[0:32] = np.pad(w5[64:96], ((0, 0), (1, 1), (1, 1))).reshape(32, 49)
    wB[32:128] = w7.reshape(96, 49)
    c["crpeB_pack"] = _pack_diag(wB, 49)
    return c


# ------------------------------------------------------------------- device --

def build(n_batch=2, reps=1):
    import concourse.tile as tile
    from concourse import bacc, mybir
    from concourse.masks import make_identity

    F = mybir.dt.float32
    BF = mybir.dt.bfloat16
    AL = mybir.AluOpType
    AF = mybir.ActivationFunctionType

    nc = bacc.Bacc(None, target_bir_lowering=False)

    d = {}
    d["xT"] = nc.dram_tensor("xT", (n_batch, 2, P, N), F, kind="ExternalInput")
    d["out"] = nc.dram_tensor("out", (n_batch, 2, P, N), F, kind="ExternalOutput")
    for nm, sh in [("wqT", (2, P, 256)), ("wkvT", (2, P, 512)),
                   ("projT", (2, P, 256)), ("fc1T", (2, P, 1024)),
                   ("fc2T", (8, P, 256)), ("cpe_pack0", (P, 288)),
                   ("cpe_pack1", (P, 288)), ("crpeA_pack", (P, 800)),
                   ("crpeB_pack", (P, 1568)), ("vb_row", (2, P, P))]:
        d[nm] = nc.dram_tensor(nm, sh, F, kind="ExternalInput")
    for nm in ("qb", "pb", "f1b", "f2b", "ln1_g", "ln1_b", "ln2_g", "ln2_b",
               "cpe_b", "crpe_be"):
        k = 8 if nm == "f1b" else 2
        d[nm] = nc.dram_tensor(nm, (k, P, 1), F, kind="ExternalInput")

    with tile.TileContext(nc) as tc:
        _emit(nc, tc, mybir, F, BF, AL, AF, make_identity, n_batch, d,
              reps=reps)
    nc.finalize()
    return nc


def _emit(nc, tc, mybir, F, BF, AL, AF, make_identity, n_batch, d, reps=1):
    from contextlib import ExitStack
    with ExitStack() as ctx:
        wpool = ctx.enter_context(tc.tile_pool(name="wpool", bufs=1))
        mast = ctx.enter_context(tc.tile_pool(name="mast", bufs=1))
        work = ctx.enter_context(tc.tile_pool(name="work", bufs=1))
        cpool = ctx.enter_context(tc.tile_pool(name="cpool", bufs=1))
        ps = ctx.enter_context(tc.tile_pool(name="ps", bufs=1, space="PSUM"))

        # ---- constants (staged through one f32 tile, kept bf16) ----
        def cbf(name, src, shape):
            st = wpool.tile([P, 1600], F, name=f"stage_{name}", tag="stage")
            nc.sync.dma_start(out=st[:shape[0], :shape[1]], in_=src)
            t = wpool.tile(list(shape), BF, name=name, tag=name)
            nc.vector.tensor_copy(t, st[:shape[0], :shape[1]])
            return t

        K = {}
        for i in range(2):
            K[f"wq{i}"] = cbf(f"wq{i}", d["wqT"][i], (P, 256))
            K[f"wkv{i}"] = cbf(f"wkv{i}", d["wkvT"][i], (P, 512))
            K[f"wp{i}"] = cbf(f"wp{i}", d["projT"][i], (P, 256))
            K[f"wf1{i}"] = cbf(f"wf1{i}", d["fc1T"][i], (P, 1024))
            K[f"cpk{i}"] = cbf(f"cpk{i}", d[f"cpe_pack{i}"][:, :], (P, 288))
            K[f"vbr{i}"] = cbf(f"vbr{i}", d["vb_row"][i], (P, P))
        for i in range(8):
            K[f"wf2{i}"] = cbf(f"wf2{i}", d["fc2T"][i], (P, 256))
        K["crA"] = cbf("crA", d["crpeA_pack"][:, :], (P, 800))
        K["crB"] = cbf("crB", d["crpeB_pack"][:, :], (P, 1568))
        pc = {}
        for nm in ("qb", "pb", "f1b", "f2b", "ln1_g", "ln1_b", "ln2_g",
                   "ln2_b", "cpe_b", "crpe_be"):
            k = 8 if nm == "f1b" else 2
            pc[nm] = []
            for i in range(k):
                t = wpool.tile([P, 1], F, name=f"{nm}{i}", tag=f"{nm}{i}")
                nc.sync.dma_start(out=t, in_=d[nm][i])
                pc[nm].append(t)
        ones_col = wpool.tile([P, 1], BF, name="ones_col", tag="ones_col")
        nc.vector.memset(ones_col, 1.0)
        ident = wpool.tile([P, P], F, name="ident", tag="ident")
        make_identity(nc, ident)
        eps_col = wpool.tile([P, 1], F, name="eps_col", tag="eps_col")
        nc.vector.memset(eps_col, 1e-6)

        env = dict(nc=nc, mybir=mybir, F=F, BF=BF, AL=AL, AF=AF, K=K, pc=pc,
                   ones_col=ones_col, ident=ident, eps_col=eps_col,
                   wpool=wpool, mast=mast, work=work, cpool=cpool, ps=ps, d=d)
        if reps == 1:
            for b in range(n_batch):
                _one_batch(env, b)
        else:
            with tc.For_i(0, reps, 1):
                for b in range(n_batch):
                    _one_batch(env, b)


def _mm(env, out, lhsT, rhs, start, stop, tp=None):
    env["nc"].tensor.matmul(out, lhsT, rhs, start=start, stop=stop,
                            tile_position=tp, skip_group_check=True)


def _one_batch(env, b):
    import os
    STOP = int(os.environ.get("KSTOP", "99"))
    nc, F, BF, AL, AF = env["nc"], env["F"], env["BF"], env["AL"], env["AF"]
    K, pc = env["K"], env["pc"]
    mast, work, cpool, ps = env["mast"], env["work"], env["cpool"], env["ps"]
    d = env["d"]

    def bail(bufs):
        for ct in range(2):
            nc.sync.dma_start(out=d["out"][b, ct], in_=bufs[ct])
        return True

    # ---------------- load x feature-major --------------------------------
    x = [mast.tile([P, N], F, name=f"x{ct}", tag=f"x{ct}") for ct in range(2)]
    for ct in range(2):
        nc.sync.dma_start(out=x[ct], in_=d["xT"][b, ct])

    # ---------------- cpe: pad, rotate, conv, resid -----------------------
    for ct in range(2):
        pad = work.tile([P, PADBUF], BF, name=f"pad{ct}", tag=f"big{ct}")
        nc.vector.memset(pad, 0.0)
        nc.vector.tensor_copy(
            _pv(pad)[:, 3:59, 3:59],
            x[ct][:, 1:].rearrange("p (r w) -> p r w", w=HW))
        rots = _mk_rots(env, pad, ct)
        _dwconv_tile(env, [pad] + rots, K[f"cpk{ct}"], _grid(3), 9,
                     consumer=("cpe", x[ct], pc["cpe_b"][ct]))

    if STOP <= 1:
        return bail(x)

    # ---------------- LN1 -> cur ------------------------------------------
    cur = [work.tile([P, 3200], BF, name=f"cur{ct}", tag=f"cur{ct}")
           for ct in range(2)]
    _layernorm(env, x, cur, pc["ln1_g"], pc["ln1_b"])
    for ct in range(2):
        nc.vector.memset(cur[ct][:, N:3200], 0.0)

    if STOP <= 2:
        # dump cur via fp32 staging
        xs = [work.tile([P, N], F, name=f"dmp{ct}", tag=f"big{ct}") for ct in range(2)]
        for ct in range(2):
            nc.vector.tensor_copy(xs[ct], cur[ct][:, :N])
        return bail(xs)

    # ---------------- qkv ---------------------------------------------------
    q = [work.tile([P, N], BF, name=f"q{ct}", tag=f"q{ct}") for ct in range(2)]
    for ft in range(2):
        for (n0, nn) in SEQ_CHUNKS:
            pt = ps.tile([P, 512], F, name="qps", tag="mmps", bufs=2)
            for kt in range(2):
                _mm(env, pt[:, :nn], K[f"wq{kt}"][:, 128 * ft:128 * ft + 128],
                    cur[kt][:, n0:n0 + nn], kt == 0, kt == 1)
            nc.vector.tensor_scalar_add(out=q[ft][:, n0:n0 + nn],
                                        in0=pt[:, :nn], scalar1=pc["qb"][ft])
    kex = work.tile([P, NTOK_T * 256], BF, name="kex", tag="kex")
    vtm = work.tile([P, NTOK_T * 256], BF, name="vtm", tag="vtm")
    nc.vector.memset(kex, 0.0)
    nc.vector.memset(vtm, 0.0)
    for tt in range(NTOK_T):
        m = P if tt < 24 else LAST_M
        pt = ps.tile([P, 512], F, name="kvps", tag="mmps", bufs=2)
        for kt in range(2):
            _mm(env, pt, cur[kt][:, P * tt:P * tt + P], K[f"wkv{kt}"],
                kt == 0, kt == 1)
        nc.scalar.activation(out=kex[:m, 256 * tt:256 * tt + 256],
                             in_=pt[:m, 0:256], func=AF.Exp)
        nc.vector.tensor_copy(vtm[:m, 256 * tt:256 * tt + 256],
                              pt[:m, 256:512])

    if STOP <= 3:
        xs = [work.tile([P, N], F, name=f"dmp{ct}", tag=f"big{ct}") for ct in range(2)]
        for ct in range(2):
            nc.vector.tensor_copy(xs[ct], q[ct])
        return bail(xs)

    # ---------------- ksum, kv --------------------------------------------
    ksum_ps = ps.tile([P, 2], F, name="ksum_ps", tag="sps")
    for g in range(2):
        for tt in range(NTOK_T):
            _mm(env, ksum_ps[:, g:g + 1],
                kex[:, 256 * tt + 128 * g:256 * tt + 128 * g + 128],
                env["ones_col"], tt == 0, tt == 24)
    rk = work.tile([P, 2], F, name="rk", tag="rk")
    nc.vector.reciprocal(rk, ksum_ps)
    kv = [work.tile([P, P], BF, name=f"kv{g}", tag=f"kv{g}") for g in range(2)]
    for g in range(2):
        kvp = ps.tile([P, P], F, name=f"kvp{g}", tag="kvg")
        for tt in range(NTOK_T):
            _mm(env, kvp, kex[:, 256 * tt + 128 * g:256 * tt + 128 * g + 128],
                vtm[:, 256 * tt + 128 * g:256 * tt + 128 * g + 128],
                tt == 0, tt == 24)
        nc.vector.scalar_tensor_tensor(out=kv[g], in0=kvp,
                                       scalar=rk[:, g:g + 1],
                                       in1=K[f"vbr{g}"],
                                       op0=AL.mult, op1=AL.add)

    # ---------------- factor ----------------------------------------------
    fac = [work.tile([P, N], BF, name=f"fac{g}", tag=("kex", "vtm")[g])
           for g in range(2)]
    for g in range(2):
        for (n0, nn) in SEQ_CHUNKS:
            pt = ps.tile([P, 512], F, name="fps", tag="mmps", bufs=2)
            for hh in range(4):
                s = 32 * hh
                _mm(env, pt[s:s + 32, :nn], kv[g][s:s + 32, s:s + 32],
                    q[g][s:s + 32, n0:n0 + nn], True, True, tp=(s, s))
            nc.vector.tensor_copy(fac[g][:, n0:n0 + nn], pt[:, :nn])

    if STOP <= 4:
        xs = [work.tile([P, N], F, name=f"dmp{ct}", tag=f"big{ct}") for ct in range(2)]
        for ct in range(2):
            nc.vector.tensor_copy(xs[ct], fac[ct])
        return bail(xs)

    # ---------------- v^T -> padded image; crpe conv; ev; att --------------
    att = [work.tile([P, N], BF, name=f"att{ct}", tag=f"cur{ct}")
           for ct in range(2)]
    for ct in range(2):
        nc.vector.tensor_copy(att[ct][:, 0:1], fac[ct][:, 0:1])
        vpad = work.tile([P, PADBUF], BF, name=f"vpad{ct}", tag=f"big{ct}")
        nc.vector.memset(vpad, 0.0)
        for ch in range(NCHUNK):
            pt = ps.tile([P, 512], F, name="vps", tag="mmps", bufs=2)
            for kt in range(2):
                _mm(env, pt[:, :CPX],
                    K[f"wkv{kt}"][:, 256 + 128 * ct:256 + 128 * ct + 128],
                    cur[kt][:, 1 + CPX * ch:1 + CPX * ch + CPX],
                    kt == 0, kt == 1)
            nc.vector.tensor_copy(
                _pv(vpad)[:, 3 + RPC * ch:3 + RPC * ch + RPC, 3:59],
                pt[:, :CPX].rearrange("p (r w) -> p r w", w=HW))
        rots = _mk_rots(env, vpad, ct)
        _dwconv_tile(env, [vpad] + rots,
                     K["crA"] if ct == 0 else K["crB"],
                     _grid(5) if ct == 0 else _grid(7),
                     25 if ct == 0 else 49,
                     consumer=("crpe", (q[ct], fac[ct], att[ct]),
                               pc["crpe_be"][ct]))

    if STOP <= 5:
        xs = [work.tile([P, N], F, name=f"dmp{ct}", tag=f"xbf{ct}") for ct in range(2)]
        for ct in range(2):
            nc.vector.tensor_copy(xs[ct], att[ct])
        return bail(xs)

    # ---------------- proj + resid -> x2 (in place) ------------------------
    for ft in range(2):
        for (n0, nn) in SEQ_CHUNKS:
            pt = ps.tile([P, 512], F, name="pps", tag="mmps", bufs=2)
            for kt in range(2):
                _mm(env, pt[:, :nn], K[f"wp{kt}"][:, 128 * ft:128 * ft + 128],
                    att[kt][:, n0:n0 + nn], kt == 0, kt == 1)
            nc.vector.scalar_tensor_tensor(
                out=x[ft][:, n0:n0 + nn], in0=pt[:, :nn],
                scalar=pc["pb"][ft], in1=x[ft][:, n0:n0 + nn],
                op0=AL.add, op1=AL.add)

    if STOP <= 6:
        return bail(x)

    # ---------------- LN2 -> cur2 ------------------------------------------
    cur2 = [work.tile([P, 3200], BF, name=f"cur2_{ct}", tag=f"cur{ct}")
            for ct in range(2)]
    _layernorm(env, x, cur2, pc["ln2_g"], pc["ln2_b"])

    # ---------------- MLP ---------------------------------------------------
    xo = [work.tile([P, N], F, name=f"xo{ct}", tag=f"big{ct}")
          for ct in range(2)]
    for (n0, nn) in SEQ_CHUNKS:
        hb = []
        for ft in range(8):
            pt = ps.tile([P, 512], F, name="hps", tag="mmps", bufs=2)
            for kt in range(2):
                _mm(env, pt[:, :nn], K[f"wf1{kt}"][:, 128 * ft:128 * ft + 128],
                    cur2[kt][:, n0:n0 + nn], kt == 0, kt == 1)
            h = work.tile([P, 512], BF, name=f"h{ft}", tag=f"h{ft}", bufs=2)
            nc.scalar.activation(out=h[:, :nn], in_=pt[:, :nn], func=AF.Gelu,
                                 bias=pc["f1b"][ft], scale=1.0)
            hb.append(h)
        for ct in range(2):
            pt2 = ps.tile([P, 512], F, name="ops", tag="mmps", bufs=2)
            for kt in range(8):
                _mm(env, pt2[:, :nn], K[f"wf2{kt}"][:, 128 * ct:128 * ct + 128],
                    hb[kt][:, :nn], kt == 0, kt == 7)
            nc.vector.scalar_tensor_tensor(
                out=xo[ct][:, n0:n0 + nn], in0=pt2[:, :nn],
                scalar=pc["f2b"][ct], in1=x[ct][:, n0:n0 + nn],
                op0=AL.add, op1=AL.add)
    for ct in range(2):
        nc.sync.dma_start(out=d["out"][b, ct], in_=xo[ct])


def _pv(padt):
    return padt[:, GUARD:GUARD + PADIMG].rearrange("p (r w) -> p r w", w=PW)


def _mk_rots(env, pad, ct):
    nc = env["nc"]
    rots = []
    for r in range(1, 4):
        sr = env["cpool"].tile([P, PADBUF], env["BF"], name=f"rot{r}",
                               tag=f"rot{r}")
        for i in range(4):
            j = (i + r) % 4
            nc.sync.dma_start(out=sr[32 * i:32 * i + 32, :],
                              in_=pad[32 * j:32 * j + 32, :])
        rots.append(sr)
    return rots


def _dwconv_tile(env, stacks, pack, offs, T, consumer):
    """16-way depthwise conv for one 128-channel image tile."""
    nc, F, BF, AL = env["nc"], env["F"], env["BF"], env["AL"]
    ps, cpool = env["ps"], env["cpool"]
    import os
    KCONV = int(os.environ.get("KCONV", "1"))
    kind = consumer[0]
    last_t = {}
    for t in range(T):
        last_t[t % 4] = t
    for ch in range(NCHUNK):
        obase = GUARD + (3 + RPC * ch) * PW
        if KCONV == 1:
            pts = [ps.tile([P, CONV_N], F, name=f"cv{bk}", tag=f"cv{bk}")
                   for bk in range(4)]
            started = [[False] * 4 for _ in range(4)]
            for t in range(T):
                bk = t % 4
                off = offs[t]
                for jj in range(4):
                    ii = (jj - bk) % 4
                    _mm(env, pts[bk][32 * jj:32 * jj + 32, :],
                        pack[32 * ii:32 * ii + 32, 32 * t:32 * t + 32],
                        stacks[bk][32 * ii:32 * ii + 32,
                                   obase + off:obase + off + CONV_N],
                        not started[bk][jj], t == last_t[bk],
                        tp=(32 * ii, 32 * jj))
                    started[bk][jj] = True
            sb = [cpool.tile([P, CONV_N], BF, name=f"cs{bk}", tag=f"cs{bk}")
                  for bk in range(4)]
            for bk in range(4):
                nc.vector.tensor_copy(sb[bk], pts[bk])
            nc.gpsimd.tensor_add(sb[0], sb[0], sb[1])
            nc.gpsimd.tensor_add(sb[2], sb[2], sb[3])
            nc.gpsimd.tensor_add(sb[0], sb[0], sb[2])
        elif KCONV == 2:
            # permuted-block-diag full-K matmuls: one MM per tap per chunk
            pt0 = ps.tile([P, CONV_N], F, name="cv0", tag="cv0")
            for t in range(T):
                off = offs[t]
                _mm(env, pt0, pack[:, 32 * t:32 * t + 32].rearrange(
                        "p a -> p a"),
                    stacks[t % 4][:, obase + off:obase + off + CONV_N],
                    t == 0, t == T - 1)
            sb = [cpool.tile([P, CONV_N], BF, name="cs0", tag="cs0")]
            nc.vector.tensor_copy(sb[0], pt0)
        else:
            sb = [cpool.tile([P, CONV_N], BF, name="cs0", tag="cs0")]
            nc.vector.memset(sb[0], 0.0)
        sv = sb[0].rearrange("p (r w) -> p r w", w=PW)[:, :, 3:59]
        px0 = CPX * ch
        if kind == "cpe":
            _, xm, bias = consumer
            xv = xm[:, 1 + px0:1 + px0 + CPX].rearrange("p (r w) -> p r w",
                                                        w=HW)
            nc.vector.scalar_tensor_tensor(out=xv, in0=sv, scalar=bias,
                                           in1=xv, op0=AL.add, op1=AL.add)
        else:
            _, (qt, ft_, at), bias = consumer
            qv = qt[:, 1 + px0:1 + px0 + CPX].rearrange("p (r w) -> p r w",
                                                        w=HW)
            ev = cpool.tile([P, CPX], BF, name="ev", tag="ev")
            nc.vector.scalar_tensor_tensor(
                out=ev.rearrange("p (r w) -> p r w", w=HW), in0=sv,
                scalar=bias, in1=qv, op0=AL.add, op1=AL.mult)
            nc.vector.tensor_add(at[:, 1 + px0:1 + px0 + CPX], ev,
                                 ft_[:, 1 + px0:1 + px0 + CPX])


def _layernorm(env, x, curo, g_pp, b_pp):
    nc, F, BF, AL, AF = env["nc"], env["F"], env["BF"], env["AL"], env["AF"]
    work, ps = env["work"], env["ps"]
    xbf, sq = [], []
    for ct in range(2):
        xb = work.tile([P, 3200], BF, name=f"xbf{ct}", tag=f"xbf{ct}")
        nc.vector.tensor_copy(xb[:, :N], x[ct])
        nc.vector.memset(xb[:, N:3200], 0.0)
        s = work.tile([P, 3200], BF, name=f"sq{ct}", tag=f"big{ct}")
        nc.vector.tensor_mul(s, xb, xb)
        xbf.append(xb)
        sq.append(s)
    st = ps.tile([P, 64], F, name="lnstat", tag="sps")
    for tt in range(NTOK_T):
        for kt in range(2):
            _mm(env, st[:, 2 * tt:2 * tt + 1],
                xbf[kt][:, P * tt:P * tt + P], env["ones_col"],
                kt == 0, kt == 1)
            _mm(env, st[:, 2 * tt + 1:2 * tt + 2],
                sq[kt][:, P * tt:P * tt + P], env["ones_col"],
                kt == 0, kt == 1)
    stv = st.rearrange("p (t two) -> p t two", two=2)
    mu = work.tile([P, NTOK_T], F, name="mu", tag="mu")
    nc.vector.tensor_scalar_mul(out=mu, in0=stv[:, 0:NTOK_T, 0],
                                scalar1=1.0 / C)
    var = work.tile([P, NTOK_T], F, name="var", tag="var")
    nc.vector.tensor_scalar_mul(out=var, in0=stv[:, 0:NTOK_T, 1],
                                scalar1=1.0 / C)
    mu2 = work.tile([P, NTOK_T], F, name="mu2", tag="mu2")
    nc.vector.tensor_mul(mu2, mu, mu)
    nc.vector.tensor_sub(var, var, mu2)
    # rstd = exp(-0.5 * ln(var + eps))   (stays on the ln/exp table set)
    nc.scalar.activation(out=var, in_=var, func=AF.Ln, bias=env["eps_col"],
                         scale=1.0)
    rstd = work.tile([P, NTOK_T], F, name="rstd", tag="rstd")
    nc.scalar.activation(out=rstd, in_=var, func=AF.Exp, bias=0.0, scale=-0.5)
    negd = work.tile([P, NTOK_T], F, name="negd", tag="negd")
    nc.vector.tensor_mul(negd, mu, rstd)
    nc.vector.tensor_scalar_mul(out=negd, in0=negd, scalar1=-1.0)
    pk = work.tile([P, 64], F, name="lnpk", tag="lnpk")
    nc.vector.memset(pk, 0.0)
    nc.vector.tensor_copy(pk[:, 0:NTOK_T], rstd)
    nc.vector.tensor_copy(pk[:, 32:32 + NTOK_T], negd)
    tp = ps.tile([P, P], F, name="lntp", tag="kvg")
    nc.tensor.transpose(tp[0:64, :], pk, env["ident"])
    tps = work.tile([64, P], BF, name="lntps", tag="lntps")
    nc.vector.tensor_copy(tps, tp[0:64, :])
    rows = [work.tile([1, 3200], BF, name=f"lnrow{i}", tag=f"lnrow{i}")
            for i in range(2)]
    nc.sync.dma_start(
        out=rows[0].rearrange("o (t p) -> o t p", p=P), in_=tps[0:NTOK_T, :])
    nc.sync.dma_start(
        out=rows[1].rearrange("o (t p) -> o t p", p=P),
        in_=tps[32:32 + NTOK_T, :])
    rbc = work.tile([P, 3200], BF, name="rbc", tag="q0")
    dbc = work.tile([P, 3200], BF, name="dbc", tag="q1")
    nc.gpsimd.partition_broadcast(rbc, rows[0])
    nc.gpsimd.partition_broadcast(dbc, rows[1])
    for ct in range(2):
        d2 = work.tile([P, N], BF, name=f"lnd2_{ct}", tag=f"big{ct}")
        nc.vector.scalar_tensor_tensor(out=d2, in0=dbc[:, :N],
                                       scalar=g_pp[ct],
                                       in1=b_pp[ct].to_broadcast((P, N)),
                                       op0=AL.mult, op1=AL.add)
        nc.vector.scalar_tensor_tensor(out=curo[ct][:, :N],
                                       in0=xbf[ct][:, :N],
                                       scalar=g_pp[ct], in1=rbc[:, :N],
                                       op0=AL.mult, op1=AL.mult)
        nc.vector.tensor_add(curo[ct][:, :N], curo[ct][:, :N], d2)


# --------------------------------------------------------------- entrypoint --

def kernel(**inputs):
    from concourse.bass_utils import run_bass_kernel_spmd

    x = np.asarray(inputs["x"], np.float32)
    assert int(inputs["H"]) == HW and int(inputs["W"]) == HW
    consts = _prep_consts({k: np.asarray(v, np.float32)
                           for k, v in inputs.items()
                           if k not in ("x", "H", "W")})

    if "main" not in _COMPILED:
        _COMPILED["main"] = build(n_batch=2)
    nc = _COMPILED["main"]

    n_cores = 8
    per = B // n_cores
    base = {nm: np.ascontiguousarray(consts[nm], np.float32) for nm in (
        "wqT", "wkvT", "projT", "fc1T", "fc2T", "cpe_pack0", "cpe_pack1",
        "crpeA_pack", "crpeB_pack", "vb_row", "qb", "pb", "f1b", "f2b",
        "ln1_g", "ln1_b", "ln2_g", "ln2_b", "cpe_b", "crpe_be")}

    in_maps = []
    for c in range(n_cores):
        xb = x[c * per:(c + 1) * per]
        m = dict(base)
        m["xT"] = np.ascontiguousarray(xb.transpose(0, 2, 1)).reshape(
            per, 2, P, N)
        in_maps.append(m)

    global _last_in_maps
    _last_in_maps = in_maps
    res = run_bass_kernel_spmd(nc, in_maps, list(range(n_cores))).results
    outs = [res[c]["out"].reshape(per, C, N).transpose(0, 2, 1)
            for c in range(n_cores)]
    return np.concatenate(outs, axis=0).astype(np.float32)



# revision 5
# speedup vs baseline: 6717.8591x; 6717.8591x over previous
"""CoaT serial block (ConvPosEnc + FactorAtt-ConvRelPosEnc + MLP) on 8 trn2
NeuronCores, data-parallel over batch (16 -> 2 per core).

Per-core layout strategy (v2):
  - Master activations FEATURE-major: x^T [C=256 (2 part-tiles), N=3137] fp32;
    matmul operands bf16.
  - LayerNorm gains/biases folded into qkv/fc1 weights at host prep; LN on
    device reduces to stats (N=1 matmuls), rstd/-mu*rstd broadcast rows
    (gpsimd partition_broadcast), and two DVE tensor_tensor passes
    cur = xb*rbc + dbc.  bf16 cast and squares run on ACT.
  - Depthwise convs (cpe 3x3 on x; crpe 3/5/7 on v) on the tensor engine:
    per channel-block j a replica tile R_j holds 4 column-shifted copies of
    the block image (partition group g = shift by g columns).  One matmul
    per (block, dy, dx-batch) contracts K=128 = 4 dx taps at once, at column
    strip 32j (tile_position), all accumulating into ONE psum bank.
    Consumers read the bank directly -- no bank-sum pass.
  - softmax(k) over tokens skips the max (|k| small): exp on ACT, sums via
    N=1 matmuls; kv/factor are small per-head matmuls (diagonal tiles).
  - scale=Ch^-0.5 folded into Wq; crpe weights divided by scale; k-bias
    dropped (softmax-invariant); v-bias folded into kv rows + conv bias.
  - PSUM evacuations for q/k/v/factor on ACT (Copy/Exp with bias); proj and
    fc2 residual adds on DVE scalar_tensor_tensor in place into x.
"""

import numpy as np

B, N, C, HEADS = 16, 3137, 256, 8
CH = C // HEADS
HW = 56
NPIX = HW * HW            # 3136
PW = 62                   # padded image width/height (pad=3)
GUARD = 8                 # zero cols before/after so tap reads stay in-tile
PADIMG = PW * PW          # 3844
PADBUF = PADIMG + 2 * GUARD
P = 128
NTOK_T = 25
LAST_M = N - 24 * P       # 65
SCALE = CH ** -0.5
RPC = 8                   # image rows per conv chunk
NCHUNK = HW // RPC        # 7
CONV_N = RPC * PW         # 496
CPX = RPC * HW            # 448 valid pixels per conv chunk
SEQ_CHUNKS = [(i * 512, min(512, N - i * 512)) for i in range((N + 511) // 512)]

_COMPILED = {}


# ---------------------------------------------------------------- host prep --

def _conv_descs(windows):
    """windows: per strip j the (k, wmat[32, k, k]) depthwise weights.
    Returns (descs, pack): descs[j] = list of (row, cb, ci); pack [128, 32*n]
    with column block ci holding the [128(g,c), 32] stationary for that MM."""
    cols = []
    descs = [[] for _ in range(4)]
    for j, (k, wmat) in enumerate(windows):
        p = k // 2
        batches = [(-p, range(0, min(4, k)))]
        if k > 4:
            batches.append((4 - p, range(4, k)))
        for dy in range(k):
            row = dy - p
            for cb, dxs in batches:
                blk = np.zeros((P, 32), np.float32)
                for g, dx in enumerate(dxs):
                    blk[32 * g + np.arange(32), np.arange(32)] = wmat[:, dy, dx]
                ci = len(cols)
                cols.append(blk)
                descs[j].append((row, cb, ci))
    pack = np.concatenate(cols, axis=1) if cols else np.zeros((P, 32), np.float32)
    return descs, pack


def _prep_consts(w):
    c = {}
    qkv_w = w["qkv_w"] * w["ln1_g"][None, :]
    qkv_b = w["qkv_b"] + w["qkv_w"] @ w["ln1_b"]
    c["wqT"] = np.ascontiguousarray((qkv_w[0:256] * SCALE).T).reshape(2, P, 256)
    c["wkvT"] = np.ascontiguousarray(qkv_w[256:768].T).reshape(2, P, 512)
    c["qb"] = (qkv_b[0:256] * SCALE).reshape(2, P, 1)
    vb = qkv_b[512:768]
    c["vb_row"] = np.stack([np.broadcast_to(vb[128 * g:128 * g + 128], (P, P))
                            for g in range(2)]).copy()
    c["projT"] = np.ascontiguousarray(w["proj_w"].T).reshape(2, P, 256)
    c["pb"] = w["proj_b"].reshape(2, P, 1)
    fc1_w = w["fc1_w"] * w["ln2_g"][None, :]
    fc1_b = w["fc1_b"] + w["fc1_w"] @ w["ln2_b"]
    c["fc1T"] = np.ascontiguousarray(fc1_w.T).reshape(2, P, 1024)
    c["f1b"] = fc1_b.reshape(8, P, 1)
    c["fc2T"] = np.ascontiguousarray(w["fc2_w"].T).reshape(8, P, 256)
    c["f2b"] = w["fc2_b"].reshape(2, P, 1)
    c["cpe_b"] = w["cpe_b"].reshape(2, P, 1)
    crpe_b_cat = np.concatenate([w["crpe_b3"], w["crpe_b5"], w["crpe_b7"]])
    wsum = np.concatenate([
        w["crpe_w3"].reshape(64, -1).sum(1),
        w["crpe_w5"].reshape(96, -1).sum(1),
        w["crpe_w7"].reshape(96, -1).sum(1)])
    c["crpe_be"] = ((crpe_b_cat + vb * wsum) / SCALE).reshape(2, P, 1)

    # conv descriptors + packs
    cw = w["cpe_w"][:, 0]                       # [256, 3, 3]
    w3 = w["crpe_w3"][:, 0] / SCALE             # [64, 3, 3]
    w5 = w["crpe_w5"][:, 0] / SCALE             # [96, 5, 5]
    w7 = w["crpe_w7"][:, 0] / SCALE             # [96, 7, 7]
    meta = {}
    for ct in range(2):
        wins = [(3, cw[ct * P + 32 * j:ct * P + 32 * j + 32]) for j in range(4)]
        meta[f"cpe{ct}"], c[f"cpe_pack{ct}"] = _conv_descs(wins)
    winsA = [(3, w3[0:32]), (3, w3[32:64]), (5, w5[0:32]), (5, w5[32:64])]
    meta["crpe0"], c["crpe_pack0"] = _conv_descs(winsA)
    winsB = [(5, w5[64:96]), (7, w7[0:32]), (7, w7[32:64]), (7, w7[64:96])]
    meta["crpe1"], c["crpe_pack1"] = _conv_descs(winsB)
    return c, meta


# ------------------------------------------------------------------- device --

def build(n_batch=2, reps=1, meta=None):
    import concourse.tile as tile
    from concourse import bacc, mybir
    from concourse.masks import make_identity

    F = mybir.dt.float32
    BF = mybir.dt.bfloat16
    AL = mybir.AluOpType
    AF = mybir.ActivationFunctionType

    if meta is None:
        # descriptor structure depends only on window sizes, not values
        _, meta = _prep_consts(_dummy_weights())

    nc = bacc.Bacc(None, target_bir_lowering=False)

    d = {}
    d["xT"] = nc.dram_tensor("xT", (n_batch, 2, P, N), F, kind="ExternalInput")
    d["out"] = nc.dram_tensor("out", (n_batch, 2, P, N), F, kind="ExternalOutput")
    packs = [("wqT", (2, P, 256)), ("wkvT", (2, P, 512)),
             ("projT", (2, P, 256)), ("fc1T", (2, P, 1024)),
             ("fc2T", (8, P, 256)), ("vb_row", (2, P, P))]
    for ct in range(2):
        packs.append((f"cpe_pack{ct}", (P, 32 * 12)))
        npk = sum(len(meta[f"crpe{ct}"][j]) for j in range(4))
        packs.append((f"crpe_pack{ct}", (P, 32 * npk)))
    for nm, sh in packs:
        d[nm] = nc.dram_tensor(nm, sh, F, kind="ExternalInput")
    for nm in ("qb", "pb", "f1b", "f2b", "cpe_b", "crpe_be"):
        k = 8 if nm == "f1b" else 2
        d[nm] = nc.dram_tensor(nm, (k, P, 1), F, kind="ExternalInput")

    with tile.TileContext(nc) as tc:
        _emit(nc, tc, mybir, F, BF, AL, AF, make_identity, n_batch, d, meta,
              reps=reps)
    nc.finalize()
    return nc


def _dummy_weights():
    z = np.zeros
    return {
        "cpe_w": z((C, 1, 3, 3), np.float32), "cpe_b": z((C,), np.float32),
        "ln1_g": np.ones((C,), np.float32), "ln1_b": z((C,), np.float32),
        "qkv_w": z((3 * C, C), np.float32), "qkv_b": z((3 * C,), np.float32),
        "crpe_w3": z((64, 1, 3, 3), np.float32), "crpe_b3": z((64,), np.float32),
        "crpe_w5": z((96, 1, 5, 5), np.float32), "crpe_b5": z((96,), np.float32),
        "crpe_w7": z((96, 1, 7, 7), np.float32), "crpe_b7": z((96,), np.float32),
        "proj_w": z((C, C), np.float32), "proj_b": z((C,), np.float32),
        "ln2_g": np.ones((C,), np.float32), "ln2_b": z((C,), np.float32),
        "fc1_w": z((4 * C, C), np.float32), "fc1_b": z((4 * C,), np.float32),
        "fc2_w": z((C, 4 * C), np.float32), "fc2_b": z((C,), np.float32),
    }


def _emit(nc, tc, mybir, F, BF, AL, AF, make_identity, n_batch, d, meta,
          reps=1):
    from contextlib import ExitStack
    with ExitStack() as ctx:
        wpool = ctx.enter_context(tc.tile_pool(name="wpool", bufs=1))
        mast = ctx.enter_context(tc.tile_pool(name="mast", bufs=1))
        work = ctx.enter_context(tc.tile_pool(name="work", bufs=1))
        cpool = ctx.enter_context(tc.tile_pool(name="cpool", bufs=1))
        ps = ctx.enter_context(tc.tile_pool(name="ps", bufs=1, space="PSUM"))

        # ---- constants (staged through one f32 tile, kept bf16) ----
        def cbf(name, src, shape):
            st = wpool.tile([P, 2048], F, name=f"stage_{name}", tag="stage")
            nc.sync.dma_start(out=st[:shape[0], :shape[1]], in_=src)
            t = wpool.tile(list(shape), BF, name=name, tag=name)
            nc.vector.tensor_copy(t, st[:shape[0], :shape[1]])
            return t

        K = {}
        for i in range(2):
            K[f"wq{i}"] = cbf(f"wq{i}", d["wqT"][i], (P, 256))
            K[f"wkv{i}"] = cbf(f"wkv{i}", d["wkvT"][i], (P, 512))
            K[f"wp{i}"] = cbf(f"wp{i}", d["projT"][i], (P, 256))
            K[f"wf1{i}"] = cbf(f"wf1{i}", d["fc1T"][i], (P, 1024))
            K[f"vbr{i}"] = cbf(f"vbr{i}", d["vb_row"][i], (P, P))
            K[f"cpk{i}"] = cbf(f"cpk{i}", d[f"cpe_pack{i}"][:, :], (P, 32 * 12))
            npk = sum(len(meta[f"crpe{i}"][j]) for j in range(4))
            K[f"crk{i}"] = cbf(f"crk{i}", d[f"crpe_pack{i}"][:, :], (P, 32 * npk))
        for i in range(8):
            K[f"wf2{i}"] = cbf(f"wf2{i}", d["fc2T"][i], (P, 256))
        pc = {}
        for nm in ("qb", "pb", "f1b", "f2b", "cpe_b", "crpe_be"):
            k = 8 if nm == "f1b" else 2
            pc[nm] = []
            for i in range(k):
                t = wpool.tile([P, 1], F, name=f"{nm}{i}", tag=f"{nm}{i}")
                nc.sync.dma_start(out=t, in_=d[nm][i])
                pc[nm].append(t)
        ones_col = wpool.tile([P, 1], BF, name="ones_col", tag="ones_col")
        nc.vector.memset(ones_col, 1.0)
        ident = wpool.tile([P, P], F, name="ident", tag="ident")
        make_identity(nc, ident)
        eps_col = wpool.tile([P, 1], F, name="eps_col", tag="eps_col")
        nc.vector.memset(eps_col, 1e-6)

        # ---- one-time zero stages: S tiles, R replicas, cur tails ----
        S = [cpool.tile([P, PADBUF], BF, name=f"S{i}", tag=f"S{i}")
             for i in range(2)]
        R = [cpool.tile([P, PADBUF], BF, name=f"R{j}", tag=f"R{j}")
             for j in range(4)]
        for t in S + R:
            nc.vector.memset(t, 0.0)
        for ct in range(2):
            cur0 = work.tile([P, 3200], BF, name=f"cur_init{ct}",
                             tag=f"cur{ct}")
            nc.vector.memset(cur0[:, N:3200], 0.0)

        env = dict(nc=nc, mybir=mybir, F=F, BF=BF, AL=AL, AF=AF, K=K, pc=pc,
                   ones_col=ones_col, ident=ident, eps_col=eps_col, meta=meta,
                   wpool=wpool, mast=mast, work=work, cpool=cpool, ps=ps, d=d,
                   S=S, R=R)
        if reps == 1:
            for b in range(n_batch):
                _one_batch(env, b)
        else:
            with tc.For_i(0, reps, 1):
                for b in range(n_batch):
                    _one_batch(env, b)


def _mm(env, out, lhsT, rhs, start, stop, tp=None):
    env["nc"].tensor.matmul(out, lhsT, rhs, start=start, stop=stop,
                            tile_position=tp, skip_group_check=True)


def _sv(t):
    return t[:, GUARD:GUARD + PADIMG].rearrange("p (r w) -> p r w", w=PW)


def _replicas(env, s):
    """R[j] partition group g <- S block j shifted by g columns."""
    nc, R = env["nc"], env["R"]
    for j in range(4):
        for g in range(4):
            nc.sync.dma_start(
                out=R[j][32 * g:32 * g + 32, 0:PADBUF - 3],
                in_=s[32 * j:32 * j + 32, g:PADBUF - 3 + g])


def _dwconv(env, descs, pack, consumer):
    """Depthwise conv over one 128-channel image (in env["R"]), one psum
    bank per chunk; consumer(pt, ch) handles chunk output."""
    nc, F, ps, R = env["nc"], env["F"], env["ps"], env["R"]
    nmax = max(len(descs[j]) for j in range(4))
    for ch in range(NCHUNK):
        obase = GUARD + (3 + RPC * ch) * PW
        pt = ps.tile([P, CONV_N], F, name="cvp", tag="cvp", bufs=2)
        for w in range(nmax):
            for j in range(4):
                if w < len(descs[j]):
                    row, cb, ci = descs[j][w]
                    base = obase + row * PW + cb
                    _mm(env, pt[32 * j:32 * j + 32, :],
                        pack[:, 32 * ci:32 * ci + 32],
                        R[j][:, base:base + CONV_N],
                        w == 0, w == len(descs[j]) - 1, tp=(0, 32 * j))
        consumer(pt, ch)


def _one_batch(env, b):
    import os
    STOP = int(os.environ.get("KSTOP", "99"))
    nc, F, BF, AL, AF = env["nc"], env["F"], env["BF"], env["AL"], env["AF"]
    K, pc, meta = env["K"], env["pc"], env["meta"]
    mast, work, cpool, ps = env["mast"], env["work"], env["cpool"], env["ps"]
    d, S = env["d"], env["S"]

    def bail(bufs):
        for ct in range(2):
            nc.sync.dma_start(out=d["out"][b, ct], in_=bufs[ct])
        return True

    # ---------------- load x feature-major --------------------------------
    x = [mast.tile([P, N], F, name=f"x{ct}", tag=f"x{ct}") for ct in range(2)]
    for ct in range(2):
        nc.sync.dma_start(out=x[ct], in_=d["xT"][b, ct])

    # ---------------- cpe: stage, replicate, conv, resid ------------------
    for ct in range(2):
        nc.scalar.activation(
            out=_sv(S[ct])[:, 3:59, 3:59],
            in_=x[ct][:, 1:].rearrange("p (r w) -> p r w", w=HW),
            func=AF.Copy)
    for ct in range(2):
        _replicas(env, S[ct])

        def cpe_consume(pt, ch, ct=ct):
            px0 = CPX * ch
            xv = x[ct][:, 1 + px0:1 + px0 + CPX].rearrange(
                "p (r w) -> p r w", w=HW)
            nc.vector.scalar_tensor_tensor(
                out=xv, in0=pt.rearrange("p (r w) -> p r w", w=PW)[:, :, 3:59],
                scalar=pc["cpe_b"][ct], in1=xv, op0=AL.add, op1=AL.add)

        _dwconv(env, meta[f"cpe{ct}"], K[f"cpk{ct}"], cpe_consume)

    if STOP <= 1:
        return bail(x)

    # ---------------- LN1 -> cur ------------------------------------------
    cur = [work.tile([P, 3200], BF, name=f"cur{ct}", tag=f"cur{ct}")
           for ct in range(2)]
    _layernorm(env, x, cur)

    if STOP <= 2:
        xs = [work.tile([P, N], F, name=f"dmp{ct}", tag=f"big{ct}")
              for ct in range(2)]
        for ct in range(2):
            nc.vector.tensor_copy(xs[ct], cur[ct][:, :N])
        return bail(xs)

    # ---------------- qkv ---------------------------------------------------
    q = [work.tile([P, N], BF, name=f"q{ct}", tag=f"q{ct}") for ct in range(2)]
    for ft in range(2):
        for (n0, nn) in SEQ_CHUNKS:
            pt = ps.tile([P, 512], F, name="qps", tag="mmps", bufs=4)
            for kt in range(2):
                _mm(env, pt[:, :nn], K[f"wq{kt}"][:, 128 * ft:128 * ft + 128],
                    cur[kt][:, n0:n0 + nn], kt == 0, kt == 1)
            nc.scalar.activation(out=q[ft][:, n0:n0 + nn], in_=pt[:, :nn],
                                 func=AF.Identity, bias=pc["qb"][ft])
    kex = work.tile([P, NTOK_T * 256], BF, name="kex", tag="kex")
    vtm = work.tile([P, NTOK_T * 256], BF, name="vtm", tag="vtm")
    nc.vector.memset(kex, 0.0)
    nc.vector.memset(vtm, 0.0)
    for tt in range(NTOK_T):
        m = P if tt < 24 else LAST_M
        pt = ps.tile([P, 512], F, name="kvps", tag="mmps", bufs=4)
        for kt in range(2):
            _mm(env, pt, cur[kt][:, P * tt:P * tt + P], K[f"wkv{kt}"],
                kt == 0, kt == 1)
        nc.scalar.activation(out=kex[:m, 256 * tt:256 * tt + 256],
                             in_=pt[:m, 0:256], func=AF.Exp)
        nc.scalar.activation(out=vtm[:m, 256 * tt:256 * tt + 256],
                             in_=pt[:m, 256:512], func=AF.Copy)

    if STOP <= 3:
        xs = [work.tile([P, N], F, name=f"dmp{ct}", tag=f"big{ct}")
              for ct in range(2)]
        for ct in range(2):
            nc.vector.tensor_copy(xs[ct], q[ct])
        return bail(xs)

    # ---------------- ksum, kv --------------------------------------------
    ksum_ps = ps.tile([P, 2], F, name="ksum_ps", tag="sps")
    for g in range(2):
        for tt in range(NTOK_T):
            _mm(env, ksum_ps[:, g:g + 1],
                kex[:, 256 * tt + 128 * g:256 * tt + 128 * g + 128],
                env["ones_col"], tt == 0, tt == 24)
    rk = work.tile([P, 2], F, name="rk", tag="rk")
    nc.vector.reciprocal(rk, ksum_ps)
    kv = [work.tile([P, P], BF, name=f"kv{g}", tag=f"kv{g}") for g in range(2)]
    for g in range(2):
        kvp = ps.tile([P, P], F, name=f"kvp{g}", tag="kvg")
        for tt in range(NTOK_T):
            _mm(env, kvp, kex[:, 256 * tt + 128 * g:256 * tt + 128 * g + 128],
                vtm[:, 256 * tt + 128 * g:256 * tt + 128 * g + 128],
                tt == 0, tt == 24)
        nc.vector.scalar_tensor_tensor(out=kv[g], in0=kvp,
                                       scalar=rk[:, g:g + 1],
                                       in1=K[f"vbr{g}"],
                                       op0=AL.mult, op1=AL.add)

    # ---------------- factor ----------------------------------------------
    fac = [work.tile([P, N], BF, name=f"fac{g}", tag=("kex", "vtm")[g])
           for g in range(2)]
    for g in range(2):
        for (n0, nn) in SEQ_CHUNKS:
            pt = ps.tile([P, 512], F, name="fps", tag="mmps", bufs=4)
            for hh in range(4):
                s = 32 * hh
                _mm(env, pt[s:s + 32, :nn], kv[g][s:s + 32, s:s + 32],
                    q[g][s:s + 32, n0:n0 + nn], True, True, tp=(s, s))
            nc.scalar.activation(out=fac[g][:, n0:n0 + nn], in_=pt[:, :nn],
                                 func=AF.Copy)

    if STOP <= 4:
        xs = [work.tile([P, N], F, name=f"dmp{ct}", tag=f"big{ct}")
              for ct in range(2)]
        for ct in range(2):
            nc.vector.tensor_copy(xs[ct], fac[ct])
        return bail(xs)

    # ---------------- v image; crpe conv; ev; att --------------------------
    att = [work.tile([P, N], BF, name=f"att{ct}", tag=f"cur{ct}")
           for ct in range(2)]
    for ct in range(2):
        nc.vector.tensor_copy(att[ct][:, 0:1], fac[ct][:, 0:1])
        for ch in range(NCHUNK):
            pt = ps.tile([P, 512], F, name="vps", tag="mmps", bufs=4)
            for kt in range(2):
                _mm(env, pt[:, :CPX],
                    K[f"wkv{kt}"][:, 256 + 128 * ct:256 + 128 * ct + 128],
                    cur[kt][:, 1 + CPX * ch:1 + CPX * ch + CPX],
                    kt == 0, kt == 1)
            nc.scalar.activation(
                out=_sv(S[ct])[:, 3 + RPC * ch:3 + RPC * ch + RPC, 3:59],
                in_=pt[:, :CPX].rearrange("p (r w) -> p r w", w=HW),
                func=AF.Copy)
    for ct in range(2):
        _replicas(env, S[ct])
        ev = cpool.tile([P, N - 1], BF, name=f"ev{ct}", tag=f"ev{ct}")

        def crpe_consume(pt, ch, ct=ct, ev=ev):
            px0 = CPX * ch
            qv = q[ct][:, 1 + px0:1 + px0 + CPX].rearrange(
                "p (r w) -> p r w", w=HW)
            nc.vector.scalar_tensor_tensor(
                out=ev[:, px0:px0 + CPX].rearrange("p (r w) -> p r w", w=HW),
                in0=pt.rearrange("p (r w) -> p r w", w=PW)[:, :, 3:59],
                scalar=pc["crpe_be"][ct], in1=qv, op0=AL.add, op1=AL.mult)
            nc.gpsimd.tensor_add(att[ct][:, 1 + px0:1 + px0 + CPX],
                                 ev[:, px0:px0 + CPX],
                                 fac[ct][:, 1 + px0:1 + px0 + CPX])

        _dwconv(env, meta[f"crpe{ct}"], K[f"crk{ct}"], crpe_consume)

    if STOP <= 5:
        xs = [work.tile([P, N], F, name=f"dmp{ct}", tag=f"big{ct}")
              for ct in range(2)]
        for ct in range(2):
            nc.vector.tensor_copy(xs[ct], att[ct])
        return bail(xs)

    # ---------------- proj + resid -> x (in place) ------------------------
    for ft in range(2):
        for (n0, nn) in SEQ_CHUNKS:
            pt = ps.tile([P, 512], F, name="pps", tag="mmps", bufs=4)
            for kt in range(2):
                _mm(env, pt[:, :nn], K[f"wp{kt}"][:, 128 * ft:128 * ft + 128],
                    att[kt][:, n0:n0 + nn], kt == 0, kt == 1)
            nc.vector.scalar_tensor_tensor(
                out=x[ft][:, n0:n0 + nn], in0=pt[:, :nn],
                scalar=pc["pb"][ft], in1=x[ft][:, n0:n0 + nn],
                op0=AL.add, op1=AL.add)

    if STOP <= 6:
        return bail(x)

    # ---------------- LN2 -> cur2 ------------------------------------------
    cur2 = [work.tile([P, 3200], BF, name=f"cur2_{ct}", tag=f"cur{ct}")
            for ct in range(2)]
    _layernorm(env, x, cur2)

    # ---------------- MLP (resid in place into x) --------------------------
    for (n0, nn) in SEQ_CHUNKS:
        hb = []
        for ft in range(8):
            pt = ps.tile([P, 512], F, name="hps", tag="mmps", bufs=4)
            for kt in range(2):
                _mm(env, pt[:, :nn], K[f"wf1{kt}"][:, 128 * ft:128 * ft + 128],
                    cur2[kt][:, n0:n0 + nn], kt == 0, kt == 1)
            h = work.tile([P, 512], BF, name=f"h{ft}", tag=f"h{ft}", bufs=2)
            nc.scalar.activation(out=h[:, :nn], in_=pt[:, :nn], func=AF.Gelu,
                                 bias=pc["f1b"][ft], scale=1.0)
            hb.append(h)
        for ct in range(2):
            pt2 = ps.tile([P, 512], F, name="ops", tag="mmps", bufs=4)
            for kt in range(8):
                _mm(env, pt2[:, :nn], K[f"wf2{kt}"][:, 128 * ct:128 * ct + 128],
                    hb[kt][:, :nn], kt == 0, kt == 7)
            nc.vector.scalar_tensor_tensor(
                out=x[ct][:, n0:n0 + nn], in0=pt2[:, :nn],
                scalar=pc["f2b"][ct], in1=x[ct][:, n0:n0 + nn],
                op0=AL.add, op1=AL.add)
    for ct in range(2):
        nc.sync.dma_start(out=d["out"][b, ct], in_=x[ct])


def _layernorm(env, x, curo):
    nc, F, BF, AL, AF = env["nc"], env["F"], env["BF"], env["AL"], env["AF"]
    work, ps = env["work"], env["ps"]
    # bf16 cast into the cur buffers (tails are zero from one-time memset)
    sq = []
    for ct in range(2):
        nc.scalar.activation(out=curo[ct][:, :N], in_=x[ct], func=AF.Copy)
        s = work.tile([P, 3200], BF, name=f"sq{ct}", tag=f"sq{ct}")
        nc.scalar.activation(out=s[:, :N], in_=curo[ct][:, :N],
                             func=AF.Square)
        nc.vector.memset(s[:, N:3200], 0.0)
        sq.append(s)
    st = ps.tile([P, 64], F, name="lnstat", tag="sps")
    for tt in range(NTOK_T):
        for kt in range(2):
            _mm(env, st[:, 2 * tt:2 * tt + 1],
                curo[kt][:, P * tt:P * tt + P], env["ones_col"],
                kt == 0, kt == 1)
            _mm(env, st[:, 2 * tt + 1:2 * tt + 2],
                sq[kt][:, P * tt:P * tt + P], env["ones_col"],
                kt == 0, kt == 1)
    stv = st.rearrange("p (t two) -> p t two", two=2)
    mu = work.tile([P, NTOK_T], F, name="mu", tag="mu")
    nc.vector.tensor_scalar_mul(out=mu, in0=stv[:, 0:NTOK_T, 0],
                                scalar1=1.0 / C)
    var = work.tile([P, NTOK_T], F, name="var", tag="var")
    nc.vector.tensor_scalar_mul(out=var, in0=stv[:, 0:NTOK_T, 1],
                                scalar1=1.0 / C)
    mu2 = work.tile([P, NTOK_T], F, name="mu2", tag="mu2")
    nc.vector.tensor_mul(mu2, mu, mu)
    nc.vector.tensor_sub(var, var, mu2)
    # rstd = exp(-0.5 * ln(var + eps))   (stays on the ln/exp table set)
    nc.scalar.activation(out=var, in_=var, func=AF.Ln, bias=env["eps_col"],
                         scale=1.0)
    rstd = work.tile([P, NTOK_T], F, name="rstd", tag="rstd")
    nc.scalar.activation(out=rstd, in_=var, func=AF.Exp, bias=0.0, scale=-0.5)
    negd = work.tile([P, NTOK_T], F, name="negd", tag="negd")
    nc.vector.tensor_mul(negd, mu, rstd)
    nc.vector.tensor_scalar_mul(out=negd, in0=negd, scalar1=-1.0)
    pk = work.tile([P, 64], F, name="lnpk", tag="lnpk")
    nc.vector.memset(pk, 0.0)
    nc.vector.tensor_copy(pk[:, 0:NTOK_T], rstd)
    nc.vector.tensor_copy(pk[:, 32:32 + NTOK_T], negd)
    tp = ps.tile([P, P], F, name="lntp", tag="kvg")
    nc.tensor.transpose(tp[0:64, :], pk, env["ident"])
    tps = work.tile([64, P], BF, name="lntps", tag="lntps")
    nc.vector.tensor_copy(tps, tp[0:64, :])
    rows = [work.tile([1, 3200], BF, name=f"lnrow{i}", tag=f"lnrow{i}")
            for i in range(2)]
    nc.sync.dma_start(
        out=rows[0].rearrange("o (t p) -> o t p", p=P), in_=tps[0:NTOK_T, :])
    nc.sync.dma_start(
        out=rows[1].rearrange("o (t p) -> o t p", p=P),
        in_=tps[32:32 + NTOK_T, :])
    rbc = work.tile([P, 3200], BF, name="rbc", tag="q0")
    dbc = work.tile([P, 3200], BF, name="dbc", tag="q1")
    nc.gpsimd.partition_broadcast(rbc, rows[0])
    nc.gpsimd.partition_broadcast(dbc, rows[1])
    for ct in range(2):
        nc.vector.tensor_mul(curo[ct], curo[ct], rbc)
        nc.vector.tensor_add(curo[ct], curo[ct], dbc)


# --------------------------------------------------------------- entrypoint --

def kernel(**inputs):
    from concourse.bass_utils import run_bass_kernel_spmd

    x = np.asarray(inputs["x"], np.float32)
    assert int(inputs["H"]) == HW and int(inputs["W"]) == HW
    consts, meta = _prep_consts({k: np.asarray(v, np.float32)
                                 for k, v in inputs.items()
                                 if k not in ("x", "H", "W")})

    if "main" not in _COMPILED:
        _COMPILED["main"] = build(n_batch=2, meta=meta)
    nc = _COMPILED["main"]

    n_cores = 8
    per = B // n_cores
    base = {nm: np.ascontiguousarray(consts[nm], np.float32) for nm in (
        "wqT", "wkvT", "projT", "fc1T", "fc2T", "cpe_pack0", "cpe_pack1",
        "crpe_pack0", "crpe_pack1", "vb_row", "qb", "pb", "f1b", "f2b",
        "cpe_b", "crpe_be")}

    in_maps = []
    for c in range(n_cores):
        xb = x[c * per:(c + 1) * per]
        m = dict(base)
        m["xT"] = np.ascontiguousarray(xb.transpose(0, 2, 1)).reshape(
            per, 2, P, N)
        in_maps.append(m)

    global _last_in_maps
    _last_in_maps = in_maps
    res = run_bass_kernel_spmd(nc, in_maps, list(range(n_cores))).results
    outs = [res[c]["out"].reshape(per, C, N).transpose(0, 2, 1)
            for c in range(n_cores)]
    return np.concatenate(outs, axis=0).astype(np.float32)
